# revision 32
# baseline (speedup 1.0000x reference)
"""MLA (DeepSeek-style multi-head latent attention) forward on 8 TRN2 NeuronCores.

Sharding: BOTH big replicated projections are sharded over sequence and
AllGathered: the shared latent ckv projection (each core computes its own
256-seq slice of [640, S] and the raw bf16 latent + locally-roped k_pe +
sum-of-squares rows are gathered first on the CC queue), and the q_down
projection (as before, split into two gathers riding mid/late phase L).
Attention and the output projection are tensor-parallel over heads (2/core);
partial wo outputs are summed on host.

RMS folding: kv_a_ln is folded into the k/v up-projection weights on host.
The latent rms scale rk (per k position) is folded into the EXP activation's
per-partition scale (scoresT has k on partitions) and into the V PSUM-drain
copy (V has k on partitions); roped k_pe is pre-multiplied by the rms on the
shard side so the exp-scale fold leaves it unscaled. No ksn materialization,
no rms broadcast matmuls on the k side.

All matmul operands are bf16 (FWL weight loads, half DMA); accumulation fp32
in PSUM. Softmax denominators accumulate on DVE and reduce across partitions
via a ones-matmul + broadcast-matmul (replacing gpsimd partition_all_reduce,
which blocked the in-order PE queue ~7us per chunk boundary). Score rope MMs
(K=64) for the two heads are emitted back-to-back so they run concurrently
in different PE row strips.

Pipeline per core (S=2048; 4 seq-chunks of 512 for full-S phases):
  C: local ckv shard [640, 256] + ssq + local k_pe rope/rms -> AG2 (first on
     the CC queue; mat depends on it).
  L: local q_down shard (as before) -> AG1a (after 6 l-tiles), AG1b (+ssq).
  M: post-AG2: DMA gathered latent/kpe/ssq into SBUF, rk prep [128,16];
     per-head k_nope (raw) and V (rk-scaled in the ACT drain).
  B: post-AG1 wq_b (12 l-tile PSUM accumulation), r_q fold, q rope.
  A: per k-block: nope MMs both heads then concurrent rope MMs; exp with
     per-partition scale SCALE*rk; causal mask on diagonal; ctx accumulation;
     denominators via ones-MM reduce + bc-MM at chunk ends; wo groups of the
     previous chunk interleaved into the score stream; bf16 outT.
Host: sum the 8 partial outT in fp32, transpose -> [1, S, HID].
"""

import numpy as np

S = 2048
HID = 2048
QLR = 1536
H_PER_CORE = 2
N_CORES = 8
NOPE = 128
ROPE = 64
VD = 128
KVL = 512
EPS = 1e-6
THETA = 10000.0
SCALE = float((NOPE + ROPE) ** -0.5)
NC_ = 4            # seq chunks
CW = 512           # chunk width
SHW = S // N_CORES  # 256-wide local shard
KB = S // 128      # 16 k-blocks
NLT = QLR // 128   # 12 l-tiles
CKR = 5 * 128 + 2  # ckv shard rows: 512 latent + 128 roped kpe + 2 ssq(f32)


def _bf16(a):
    import ml_dtypes
    return np.ascontiguousarray(np.asarray(a, np.float32)).astype(ml_dtypes.bfloat16)


def _build_program():
    import concourse.mybir as mybir
    import concourse.tile as tile
    from concourse import bacc

    f32 = mybir.dt.float32
    f32r = mybir.dt.float32r
    bf16 = mybir.dt.bfloat16
    AF = mybir.ActivationFunctionType
    OP = mybir.AluOpType

    nc = bacc.Bacc("TRN2", target_bir_lowering=False, num_devices=N_CORES)

    hid_own = nc.dram_tensor("hid_own", [HID, SHW], bf16, kind="ExternalInput")
    wqa_t = nc.dram_tensor("wqa_t", [HID, QLR], bf16, kind="ExternalInput")
    wqb_t = nc.dram_tensor("wqb_t", [QLR, 384], bf16, kind="ExternalInput")
    wkv_t = nc.dram_tensor("wkv_t", [HID, 640], bf16, kind="ExternalInput")
    wukt_d = nc.dram_tensor("wukt", [H_PER_CORE, KVL, NOPE], bf16,
                            kind="ExternalInput")
    wuv2_d = nc.dram_tensor("wuv2", [KVL, H_PER_CORE * VD], bf16,
                            kind="ExternalInput")
    wo_t = nc.dram_tensor("wo_t", [H_PER_CORE * VD, HID], bf16, kind="ExternalInput")
    cos2_d = nc.dram_tensor("cos2", [128, S], bf16, kind="ExternalInput")
    sin2n_d = nc.dram_tensor("sin2n", [128, S], bf16, kind="ExternalInput")
    cos2o_d = nc.dram_tensor("cos2o", [128, SHW], bf16, kind="ExternalInput")
    sin2o_d = nc.dram_tensor("sin2o", [128, SHW], bf16, kind="ExternalInput")
    swappb_d = nc.dram_tensor("swappb", [128, 128], bf16, kind="ExternalInput")
    maskt_d = nc.dram_tensor("maskt", [128, 128], bf16, kind="ExternalInput")
    out_t = nc.dram_tensor("out_t", [HID, S], bf16, kind="ExternalOutput")

    with tile.TileContext(nc) as tc:
        with (
            tc.tile_pool(name="stats", bufs=1) as stats,
            tc.tile_pool(name="dram", bufs=1, space="DRAM") as dram,
            tc.tile_pool(name="consts", bufs=1) as consts,
            tc.tile_pool(name="resid", bufs=1) as resid,
            tc.tile_pool(name="bqp", bufs=1) as bqp,
        ):
            ones_p = stats.tile([128, 1], f32r)
            nc.vector.memset(ones_p.bitcast(f32), 1.0)
            ones_row = stats.tile([1, 128], f32r)
            nc.vector.memset(ones_row.bitcast(f32), 1.0)
            eps_sb = stats.tile([1, 1], f32)
            nc.vector.memset(eps_sb, EPS)
            eps_p = stats.tile([128, 1], f32)
            nc.vector.memset(eps_p, EPS)

            ckb_shard = dram.tile([CKR, SHW], bf16)
            ckb_gath = dram.tile([N_CORES * CKR, SHW], bf16, addr_space="Shared")
            qdn_shard1 = dram.tile([6 * 128, SHW], bf16)
            qdn_shard2 = dram.tile([6 * 128 + 1, SHW], bf16)
            qdn_full1 = dram.tile([N_CORES * 6 * 128, SHW], bf16,
                                  addr_space="Shared")
            qdn_full2 = dram.tile([N_CORES * (6 * 128 + 1), SHW], bf16,
                                  addr_space="Shared")

            wukt_sb = consts.tile([128, H_PER_CORE, 4, NOPE], bf16)
            wuv2_sb = consts.tile([128, 4, H_PER_CORE * VD], bf16)
            cos2_sb = consts.tile([128, S], bf16)
            sin2n_sb = consts.tile([128, S], bf16)
            cos2o_sb = consts.tile([128, SHW], bf16)
            sin2o_sb = consts.tile([128, SHW], bf16)
            swappb_sb = consts.tile([128, 128], bf16)
            maskt_sb = consts.tile([128, 128], bf16)
            wqb_sb = consts.tile([128, NLT, 384], bf16)
            wo_sb = consts.tile([128, H_PER_CORE, HID], bf16)

            kpe = resid.tile([128, S], bf16)          # roped+rms k_peT (2 copies)
            kn_sb = resid.tile([128, H_PER_CORE, S], bf16)  # per-head raw k_nopeT
            v_sb = resid.tile([128, KB, H_PER_CORE * VD], bf16)  # V, rk-scaled
            ctxa = resid.tile([128, H_PER_CORE, S], bf16)
            ckb_sb = resid.tile([128, 4, S], bf16)    # gathered raw latent
            rkp = resid.tile([128, KB], f32)          # 1/rms per k position
            rkp_e = resid.tile([128, KB], f32)        # SCALE/rms (exp scale)
            qdn_all = bqp.tile([128, NLT, N_CORES, SHW], bf16)
            ssq_all = bqp.tile([1, N_CORES, SHW], bf16)

            # ========== Phase C: local ckv shard -> AG2 ==========
            hidp_ctx = tc.tile_pool(name="hidp", bufs=1)
            hidp = hidp_ctx.__enter__()
            wkvp_ctx = tc.tile_pool(name="wkvp", bufs=1)
            wkvp = wkvp_ctx.__enter__()
            cl_ctx = tc.tile_pool(name="clwork", bufs=1)
            clw = cl_ctx.__enter__()
            with (
                tc.tile_pool(name="cwork", bufs=2) as cwork,
                tc.tile_pool(name="cpsum", bufs=3, space="PSUM") as cpsum,
                tc.tile_pool(name="cp1", bufs=1, space="PSUM") as cp1,
                tc.tile_pool(name="cmisc", bufs=1, space="PSUM") as cmisc,
            ):
                hid_own_sb = hidp.tile([128, KB, SHW], bf16)
                hid_src = hid_own.ap().rearrange("(kt p) s -> p kt s", p=128)
                for q in range(4):
                    ks = slice(4 * q, 4 * q + 4)
                    nc.sync.dma_start(hid_own_sb[:, ks, :], hid_src[:, ks, :])
                wkv_sb = wkvp.tile([128, KB, 640], bf16, tag="wkv")
                for dt in range(5):
                    nc.sync.dma_start(
                        wkv_sb[:, :, 128 * dt : 128 * (dt + 1)],
                        wkv_t.ap()[:, 128 * dt : 128 * (dt + 1)].rearrange(
                            "(kt p) m -> p kt m", p=128
                        ),
                    )
                nc.sync.dma_start(cos2o_sb, cos2o_d.ap())
                nc.sync.dma_start(sin2o_sb, sin2o_d.ap())
                nc.sync.dma_start(swappb_sb, swappb_d.ap())

                ps_cssq = cp1.tile([1, SHW], f32, tag="cssq")
                ckl = [None] * 5
                ck_ps = [None] * 5

                def emit_csq(dt):
                    # square the bf16 copy on DVE (ACT's activation-table
                    # warmup stalls the chain early in the kernel)
                    sq = cwork.tile([128, SHW], f32r, tag="csq")
                    nc.vector.tensor_tensor(sq, ckl[dt], ckl[dt], OP.mult)
                    nc.tensor.matmul(
                        ps_cssq, ones_p, sq, start=(dt == 0), stop=(dt == 3)
                    )

                for dt in range(5):
                    ps_ck = cpsum.tile([128, SHW], f32, tag="ck")
                    for kt in range(KB):
                        nc.tensor.matmul(
                            ps_ck,
                            wkv_sb[:, kt, 128 * dt : 128 * (dt + 1)],
                            hid_own_sb[:, kt, :],
                            start=(kt == 0),
                            stop=(kt == KB - 1),
                        )
                    ck_ps[dt] = ps_ck
                    if dt >= 2:
                        emit_csq(dt - 2)
                    ckl[dt] = clw.tile([128, SHW], bf16, tag=f"ckl{dt}",
                                       name=f"ckl{dt}")
                    nc.vector.tensor_copy(ckl[dt], ps_ck)
                    if dt < 4:
                        nc.gpsimd.dma_start(
                            ckb_shard[128 * dt : 128 * (dt + 1), :], ckl[dt]
                        )
                emit_csq(3)

                # local rms row + roped, rms-scaled k_pe
                msk = clw.tile([1, SHW], f32, tag="msk")
                nc.scalar.activation(
                    msk, ps_cssq, AF.Sqrt, scale=1.0 / KVL, bias=eps_sb
                )
                msk_r = clw.tile([1, SHW], f32r, tag="mskr")
                nc.vector.tensor_copy(msk_r, msk)
                ssql = clw.tile([1, SHW], f32, tag="ssql")
                nc.vector.tensor_copy(ssql, ps_cssq)

                ps_sw = cmisc.tile([128, SHW], f32, tag="cm", name="ps_csw")
                nc.tensor.matmul(ps_sw, swappb_sb, ckl[4], start=True, stop=True)
                t1 = cwork.tile([128, SHW], f32, tag="ct1")
                nc.vector.tensor_tensor(t1, ckl[4], cos2o_sb, OP.mult)
                t2 = cwork.tile([128, SHW], f32, tag="ct2")
                nc.vector.tensor_tensor(t2, ps_sw, sin2o_sb, OP.mult)
                kroped = cwork.tile([128, SHW], f32, tag="ckro")
                nc.vector.tensor_tensor(kroped, t1, t2, OP.add)
                ps_bc = cmisc.tile([128, SHW], f32, tag="cm", name="ps_cbc")
                nc.tensor.matmul(ps_bc, ones_row, msk_r, start=True, stop=True)
                kpe_l = clw.tile([128, SHW], bf16, tag="kpel")
                nc.vector.tensor_tensor(kpe_l, kroped, ps_bc, OP.mult)

                nc.gpsimd.dma_start(ckb_shard[512:640, :], kpe_l)
                # ssq shipped as a hi/lo bf16 pair (~f32 precision, and all
                # valid bf16 values -- a raw f32 bitcast trips NaN checks)
                ssq_hi = clw.tile([1, SHW], bf16, tag="ssqhi")
                nc.vector.tensor_copy(ssq_hi, ssql)
                ssq_lo = clw.tile([1, SHW], bf16, tag="ssqlo")
                nc.vector.tensor_tensor(ssq_lo, ssql, ssq_hi, OP.subtract)
                nc.gpsimd.dma_start(ckb_shard[640:641, :], ssq_hi)
                nc.gpsimd.dma_start(ckb_shard[641:642, :], ssq_lo)
                nc.gpsimd.collective_compute(
                    "AllGather",
                    mybir.AluOpType.bypass,
                    replica_groups=[list(range(N_CORES))],
                    ins=[ckb_shard.opt()],
                    outs=[ckb_gath.opt()],
                )

            # ========== Phase L: local q_down shard -> split AllGather =====
            with (
                tc.tile_pool(name="wqap", bufs=4) as wqap,
                tc.tile_pool(name="lwork", bufs=2) as lwork,
                tc.tile_pool(name="lbig", bufs=1) as lbig,
                tc.tile_pool(name="lpsum", bufs=2, space="PSUM") as lpsum,
                tc.tile_pool(name="lpsum1", bufs=1, space="PSUM") as lpsum1,
            ):
                # rolling per-l-tile wqa prefetch: L streams behind the DMA
                wqa_g = [None] * NLT

                def fetch_wqa(li):
                    wqa_g[li] = wqap.tile([128, KB, 128], bf16,
                                          tag="wqa", name=f"wqa{li}")
                    nc.sync.dma_start(
                        wqa_g[li],
                        wqa_t.ap()[:, 128 * li : 128 * (li + 1)].rearrange(
                            "(kt p) m -> p kt m", p=128
                        ),
                    )

                for li in range(4):
                    fetch_wqa(li)
                # late constants, behind the phase-C/L critical DMAs
                nc.sync.dma_start(
                    wukt_sb, wukt_d.ap().rearrange("h (lt p) n -> p h lt n", p=128)
                )
                nc.sync.dma_start(
                    wuv2_sb, wuv2_d.ap().rearrange("(lt p) v -> p lt v", p=128)
                )
                for q in range(2):
                    hs_ = slice(1024 * q, 1024 * (q + 1))
                    nc.sync.dma_start(cos2_sb[:, hs_], cos2_d.ap()[:, hs_])
                    nc.sync.dma_start(sin2n_sb[:, hs_], sin2n_d.ap()[:, hs_])
                nc.sync.dma_start(maskt_sb, maskt_d.ap())
                wqb_src = wqb_t.ap().rearrange("(li p) m -> p li m", p=128)
                for q in range(3):
                    ls = slice(4 * q, 4 * q + 4)
                    nc.sync.dma_start(wqb_sb[:, ls, :], wqb_src[:, ls, :])
                wo_src = wo_t.ap().rearrange("(h p) m -> p h m", p=128)
                for q in range(4):
                    hs_ = slice(512 * q, 512 * (q + 1))
                    nc.sync.dma_start(wo_sb[:, :, hs_], wo_src[:, :, hs_])

                qdn_sb = lbig.tile([128, NLT, SHW], bf16)
                ssqb = stats.tile([1, SHW], bf16)
                ps_ssq = lpsum1.tile([1, SHW], f32, tag="ssq")

                def emit_ssq(li):
                    sq = lwork.tile([128, SHW], f32r, tag="sq")
                    nc.vector.tensor_tensor(
                        sq, qdn_sb[:, li, :], qdn_sb[:, li, :], OP.mult
                    )
                    nc.tensor.matmul(
                        ps_ssq, ones_p, sq, start=(li == 0), stop=(li == NLT - 1)
                    )

                for li in range(NLT):
                    if li + 4 < NLT:
                        fetch_wqa(li + 4)
                    ps_qd = lpsum.tile([128, SHW], f32, tag="qd",
                                       name=f"ps_qd{li % 2}")
                    for kt in range(KB):
                        nc.tensor.matmul(
                            ps_qd,
                            wqa_g[li][:, kt, :],
                            hid_own_sb[:, kt, :],
                            start=(kt == 0),
                            stop=(kt == KB - 1),
                        )
                    nc.vector.tensor_copy(qdn_sb[:, li, :], ps_qd)
                    if li > 0:
                        emit_ssq(li - 1)
                    if li == 5:
                        nc.gpsimd.dma_start(
                            qdn_shard1.rearrange("(li p) s -> p li s", p=128),
                            qdn_sb[:, 0:6, :],
                        )
                        nc.gpsimd.collective_compute(
                            "AllGather",
                            mybir.AluOpType.bypass,
                            replica_groups=[list(range(N_CORES))],
                            ins=[qdn_shard1.opt()],
                            outs=[qdn_full1.opt()],
                        )
                emit_ssq(NLT - 1)
                nc.vector.tensor_copy(ssqb, ps_ssq)
                nc.gpsimd.dma_start(
                    qdn_shard2[0 : 6 * 128, :].rearrange(
                        "(li p) s -> p li s", p=128
                    ),
                    qdn_sb[:, 6:12, :],
                )
                nc.gpsimd.dma_start(qdn_shard2[6 * 128 : 6 * 128 + 1, :], ssqb)
                nc.gpsimd.collective_compute(
                    "AllGather",
                    mybir.AluOpType.bypass,
                    replica_groups=[list(range(N_CORES))],
                    ins=[qdn_shard2.opt()],
                    outs=[qdn_full2.opt()],
                )
                src1 = qdn_full1.rearrange(
                    "(r li p) s -> p li r s", p=128, li=6
                )
                for li2 in range(6):
                    nc.gpsimd.dma_start(
                        qdn_all[:, li2, :, :], src1[:, li2, :, :]
                    )
                src2 = qdn_full2.rearrange("(r x) s -> x r s", x=6 * 128 + 1)
                nc.gpsimd.dma_start(ssq_all, src2[6 * 128 : 6 * 128 + 1, :, :])
                for li in range(6, NLT):
                    nc.gpsimd.dma_start(
                        qdn_all[:, li, :, :],
                        src2[128 * (li - 6) : 128 * (li - 5), :, :],
                    )
            wqa_g = None
            cl_ctx.__exit__(None, None, None)
            wkvp_ctx.__exit__(None, None, None)

            # ====== Phase M: unpack AG2, rk prep, k_nope + V materialize ====
            with (
                tc.tile_pool(name="mwork", bufs=2) as mwork,
                tc.tile_pool(name="kpsum", bufs=2, space="PSUM") as kpsum,
                tc.tile_pool(name="vpsum", bufs=2, space="PSUM") as vpsum,
            ):
                gview = ckb_gath.rearrange("(r x) c -> x r c", x=CKR)
                for lt in range(4):
                    nc.scalar.dma_start(
                        ckb_sb[:, lt, :].rearrange("p (r c) -> p r c", r=N_CORES),
                        gview[128 * lt : 128 * (lt + 1), :, :],
                    )
                nc.scalar.dma_start(
                    kpe.rearrange("p (r c) -> p r c", r=N_CORES),
                    gview[512:640, :, :],
                )
                # gathered ssq rows land straight in [128, KB] layout
                # (k-position on partitions): kb = 2*rank + x, p = c mod 128
                rk_hi = mwork.tile([128, KB], bf16, tag="rkhi")
                rk_lo = mwork.tile([128, KB], bf16, tag="rklo")
                for dst, srow in ((rk_hi, 640), (rk_lo, 641)):
                    dv = dst.rearrange("p (r x) -> p r x", x=2)
                    sv = gview[srow : srow + 1, :, :].rearrange(
                        "a r (x p) -> p r x", p=128
                    )
                    for x in range(2):
                        nc.scalar.dma_start(dv[:, :, x], sv[:, :, x])
                rkp_raw = mwork.tile([128, KB], f32, tag="rkpr")
                nc.vector.tensor_tensor(rkp_raw, rk_hi, rk_lo, OP.add)
                rms_p = mwork.tile([128, KB], f32, tag="rmsp")
                nc.scalar.activation(
                    rms_p, rkp_raw, AF.Sqrt, scale=1.0 / KVL, bias=eps_p
                )
                nc.vector.reciprocal_approx_fast(out=rkp, in_=rms_p)
                nc.vector.tensor_scalar_mul(rkp_e, rkp, SCALE)

                for h in range(H_PER_CORE):
                    for c in range(NC_):
                        cs = slice(CW * c, CW * (c + 1))
                        ps_k = kpsum.tile([128, CW], f32, tag="kn")
                        for lt in range(4):
                            nc.tensor.matmul(
                                ps_k,
                                wukt_sb[:, h, lt, :],
                                ckb_sb[:, lt, cs],
                                start=(lt == 0),
                                stop=(lt == 3),
                            )
                        nc.vector.tensor_copy(kn_sb[:, h, cs], ps_k)
                for b in range(KB):
                    ps_v = vpsum.tile([128, H_PER_CORE * VD], f32, tag="v")
                    for lt in range(4):
                        nc.tensor.matmul(
                            ps_v,
                            ckb_sb[:, lt, 128 * b : 128 * (b + 1)],
                            wuv2_sb[:, lt, :],
                            start=(lt == 0),
                            stop=(lt == 3),
                        )
                    nc.scalar.activation(
                        v_sb[:, b, :], ps_v, AF.Copy, scale=rkp[:, b : b + 1]
                    )

            # ===== Phase B: post-gather wq_b + q rope =====
            bres_ctx = tc.tile_pool(name="bres", bufs=1)
            bres = bres_ctx.__enter__()
            qtr = bres.tile([128, 3, S], bf16)      # post-gather q (r_q folded)
            qspe = bres.tile([128, S], bf16)        # roped q_peT
            with (
                tc.tile_pool(name="bwork", bufs=2) as bwork,
                tc.tile_pool(name="bpsum", bufs=2, space="PSUM") as bpsum,
                tc.tile_pool(name="bmisc", bufs=1, space="PSUM") as bmisc,
            ):
                rqf = bwork.tile([1, S], f32, tag="rqf")
                nc.scalar.activation(
                    rqf, ssq_all, AF.Sqrt, scale=1.0 / QLR, bias=eps_sb
                )
                nc.vector.reciprocal_approx_fast(out=rqf, in_=rqf)
                rqf_r = bwork.tile([1, S], f32r, tag="rqfr")
                nc.vector.tensor_copy(rqf_r, rqf)
                for c in range(NC_):
                    cs = slice(CW * c, CW * (c + 1))
                    ps_rq = bmisc.tile([128, CW], f32, tag="misc",
                                       name="ps_rq")
                    nc.tensor.matmul(
                        ps_rq, ones_row, rqf_r[:, cs], start=True, stop=True
                    )
                    rqsb = bwork.tile([128, CW], f32, tag="rqsb")
                    nc.vector.tensor_copy(rqsb, ps_rq)
                    for dt in range(3):
                        ps_qt = bpsum.tile([128, CW], f32, tag="qt")
                        for li in range(NLT):
                            nc.tensor.matmul(
                                ps_qt,
                                wqb_sb[:, li, 128 * dt : 128 * (dt + 1)],
                                qdn_all[:, li, 2 * c : 2 * c + 2, :],
                                start=(li == 0),
                                stop=(li == NLT - 1),
                            )
                        nc.vector.tensor_tensor(
                            qtr[:, dt, cs], ps_qt, rqsb, OP.mult
                        )
                    # rope q_pe (both heads stacked)
                    ps_sw = bmisc.tile([128, CW], f32, tag="misc",
                                       name="ps_swq")
                    nc.tensor.matmul(
                        ps_sw, swappb_sb, qtr[:, 2, cs], start=True, stop=True
                    )
                    t1 = bwork.tile([128, CW], f32, tag="t1")
                    nc.vector.tensor_tensor(
                        t1, qtr[:, 2, cs], cos2_sb[:, cs], OP.mult
                    )
                    t2 = bwork.tile([128, CW], f32, tag="t2")
                    nc.vector.tensor_tensor(t2, ps_sw, sin2n_sb[:, cs], OP.mult)
                    nc.vector.tensor_tensor(qspe[:, cs], t1, t2, OP.add)

            # ================= Phase A: attention =================
            with (
                tc.tile_pool(name="accp", bufs=2) as accp,
                tc.tile_pool(name="attp", bufs=3) as attp,
                tc.tile_pool(name="obp", bufs=2) as obp,
                tc.tile_pool(name="arow", bufs=2) as arow,
                tc.tile_pool(name="abc", bufs=2) as abc,
                tc.tile_pool(name="aacc", bufs=1, space="PSUM") as aacc,
                tc.tile_pool(name="ascore", bufs=2, space="PSUM") as ascore,
                tc.tile_pool(name="ascr1", bufs=2, space="PSUM") as ascr1,
                tc.tile_pool(name="wpsum", bufs=2, space="PSUM") as wpsum,
            ):
                def emit_w_group(c, hg):
                    cs = slice(CW * c, CW * (c + 1))
                    ob4 = obp.tile([128, 4, CW], bf16, tag="ob4")
                    for hi in range(4):
                        ht = 4 * hg + hi
                        ps_o = wpsum.tile([128, CW], f32, tag="o",
                                          name=f"ps_o{hi % 2}")
                        for h in range(H_PER_CORE):
                            nc.tensor.matmul(
                                ps_o,
                                wo_sb[:, h, 128 * ht : 128 * (ht + 1)],
                                ctxa[:, h, cs],
                                start=(h == 0),
                                stop=(h == H_PER_CORE - 1),
                            )
                        nc.vector.tensor_copy(ob4[:, hi, :], ps_o)
                    out_dst = (
                        out_t.ap()[512 * hg : 512 * (hg + 1), cs]
                        .rearrange("(ht p) s -> p ht s", p=128)
                    )
                    nc.sync.dma_start(out_dst[:, 0:2, :], ob4[:, 0:2, :])
                    nc.sync.dma_start(out_dst[:, 2:4, :], ob4[:, 2:4, :])

                chunk_state = [None]  # (c, cs, ps_ctx, rec_r)

                def emit_den(c, cs, ps_ctx, dacc):
                    rec_r = []
                    for h in range(H_PER_CORE):
                        t_den = wpsum.tile([128, CW], f32, tag="o",
                                           name=f"t_den{h}")
                        nc.tensor.matmul(
                            t_den[0:1, :], ones_p, dacc[h], start=True, stop=True
                        )
                        rec = arow.tile([1, CW], f32, tag=f"rec{h}",
                                        name=f"rec{h}")
                        nc.vector.reciprocal_approx_fast(
                            out=rec, in_=t_den[0:1, :]
                        )
                        rec_fr = arow.tile([1, CW], f32r, tag=f"recr{h}",
                                           name=f"recr{h}")
                        nc.vector.tensor_copy(rec_fr, rec)
                        rec_r.append(rec_fr)
                    chunk_state[0] = (c, cs, ps_ctx, rec_r)

                def emit_epilogue():
                    c, cs, ps_ctx, rec_r = chunk_state[0]
                    for h in range(H_PER_CORE):
                        t_bc = wpsum.tile([128, CW], f32, tag="o",
                                          name=f"t_bc{h}")
                        nc.tensor.matmul(
                            t_bc, ones_row, rec_r[h], start=True, stop=True
                        )
                        bc_sb = abc.tile([128, CW], f32, tag=f"bc{h}",
                                         name=f"bc{h}")
                        nc.scalar.activation(bc_sb, t_bc, AF.Copy)
                        nc.vector.tensor_tensor(
                            ctxa[:, h, cs], ps_ctx[h], bc_sb, OP.mult
                        )
                    chunk_state[0] = None

                for c in range(NC_):
                    cs = slice(CW * c, CW * (c + 1))
                    nj = 4 * c + 4
                    ps_ctx = [
                        aacc.tile([128, CW], f32, tag=f"ctx{h}",
                                  name=f"ps_ctx{h}")
                        for h in range(H_PER_CORE)
                    ]
                    dacc = [
                        accp.tile([128, CW], f32r, tag=f"dacc{h}",
                                  name=f"dacc{h}")
                        for h in range(H_PER_CORE)
                    ]
                    atts = [[None, None] for _ in range(nj)]
                    offs = [0 if j < 4 * c else 128 * (j - 4 * c)
                            for j in range(nj)]

                    def emit_scores(j, c=c, nj=nj, offs=offs, atts=atts,
                                    dacc=dacc):
                        off = offs[j]
                        q0 = CW * c + off
                        qs = slice(q0, CW * (c + 1))
                        pools = [ascore, ascr1]
                        ps_s = [None, None]
                        for h in range(H_PER_CORE):
                            ps_s[h] = pools[h].tile([128, CW], f32,
                                                    tag=f"sc{h}",
                                                    name=f"ps_s{h}")
                            nc.tensor.matmul(
                                ps_s[h][:, off:],
                                kn_sb[:, h, 128 * j : 128 * (j + 1)],
                                qtr[:, h, qs],
                                start=True,
                                stop=False,
                            )
                        for h in range(H_PER_CORE):
                            # K=64 rope MMs: the two heads land in PE row
                            # strips 0-63 / 64-127 and run concurrently.
                            nc.tensor.matmul(
                                ps_s[h][:, off:],
                                kpe[64 * h : 64 * (h + 1),
                                    128 * j : 128 * (j + 1)],
                                qspe[64 * h : 64 * (h + 1), qs],
                                start=False,
                                stop=True,
                            )
                        for h in range(H_PER_CORE):
                            att = attp.tile([128, CW], bf16, tag=f"att{h}",
                                            name=f"att{h}")
                            nc.scalar.activation(
                                att[:, off:], ps_s[h][:, off:], AF.Exp,
                                scale=rkp_e[:, j : j + 1],
                            )
                            if j >= 4 * c:
                                nc.vector.tensor_tensor(
                                    att[:, off : off + 128],
                                    att[:, off : off + 128],
                                    maskt_sb,
                                    OP.mult,
                                )
                            if j == 0:
                                nc.vector.tensor_copy(dacc[h], att)
                            else:
                                nc.vector.tensor_tensor(
                                    dacc[h][:, off:],
                                    dacc[h][:, off:],
                                    att[:, off:],
                                    OP.add,
                                )
                            atts[j][h] = att

                    def emit_ctx(j, c=c, nj=nj, offs=offs, atts=atts,
                                 ps_ctx=ps_ctx):
                        off = offs[j]
                        for h in range(H_PER_CORE):
                            nc.tensor.matmul(
                                ps_ctx[h][:, off:],
                                v_sb[:, j, VD * h : VD * (h + 1)],
                                atts[j][h][:, off:],
                                start=(j == 0),
                                stop=(j == nj - 1),
                            )

                    emit_scores(0)
                    emit_scores(1)
                    if chunk_state[0] is not None:
                        emit_epilogue()
                    emit_ctx(0)
                    pending_w = (
                        [(c - 1, hg) for hg in range(4)] if c >= 1 else []
                    )
                    for j in range(2, nj):
                        emit_scores(j)
                        emit_ctx(j - 1)
                        if pending_w and j >= 3:
                            emit_w_group(*pending_w.pop(0))
                    emit_ctx(nj - 1)
                    while pending_w:
                        emit_w_group(*pending_w.pop(0))
                    emit_den(c, cs, ps_ctx, dacc)
                emit_epilogue()
                for hg in range(4):
                    emit_w_group(NC_ - 1, hg)
            bres_ctx.__exit__(None, None, None)
            hidp_ctx.__exit__(None, None, None)

    nc.finalize()
    return nc


_PROGRAM = None


def _get_program():
    global _PROGRAM
    if _PROGRAM is None:
        _PROGRAM = _build_program()
    return _PROGRAM


def _host_inputs(hidden_states, position_ids, wq_a, q_a_ln_w, wq_b, wkv_a,
                 kv_a_ln_w, wkv_b, wo):
    """Build the 8 per-core input maps."""
    hs = np.asarray(hidden_states, np.float32)[0]          # [S, HID]
    pos = np.asarray(position_ids)[0].astype(np.int64)     # [S]

    # rope tables (fp32, matching the reference)
    inv_freq = (1.0 / (THETA ** (np.arange(0, ROPE, 2, dtype=np.float32) / ROPE))).astype(np.float32)
    t = pos.astype(np.float32)
    freqs = np.outer(t, inv_freq).astype(np.float32)       # [S, 32]
    emb = np.concatenate([freqs, freqs], -1)               # [S, 64]
    cos = np.cos(emb).astype(np.float32)
    sin = np.sin(emb).astype(np.float32)
    cosT = np.ascontiguousarray(cos.T)                     # [64, S]
    sinT = np.ascontiguousarray(sin.T)
    sinTn = sinT.copy()
    sinTn[:32] = -sinTn[:32]                               # fold rotate_half sign
    cos2 = np.concatenate([cosT, cosT], 0)                 # [128, S]
    sin2n = np.concatenate([sinTn, sinTn], 0)

    perm = np.concatenate([np.arange(0, ROPE, 2), np.arange(1, ROPE, 2)])  # interleave

    # swap-halves permutation matrix (two independent 64 blocks)
    swapp = np.zeros((128, 128), np.float32)
    for m in range(128):
        base = (m // 64) * 64
        i = m % 64
        swapp[base + (i + 32) % 64, m] = 1.0

    maskt = np.triu(np.ones((128, 128), np.float32))

    wq_b = np.asarray(wq_b, np.float32) * np.asarray(q_a_ln_w, np.float32)[None, :]
    kvln = np.asarray(kv_a_ln_w, np.float32)
    kvb = np.asarray(wkv_b, np.float32).reshape(16, NOPE + VD, KVL)
    wkv_a = np.asarray(wkv_a, np.float32)
    wkv_rows = np.concatenate(
        [wkv_a[:KVL], wkv_a[KVL:][perm], wkv_a[KVL:][perm]], 0
    )                                                      # [640, HID]

    hid_T = np.ascontiguousarray(hs.T)                     # [HID, S]
    shared = {
        "wqa_t": _bf16(np.asarray(wq_a, np.float32).T),
        "wkv_t": _bf16(wkv_rows.T),
        "cos2": _bf16(cos2), "sin2n": _bf16(sin2n),
        "swappb": _bf16(swapp),
        "maskt": _bf16(maskt),
    }

    wo = np.asarray(wo, np.float32)
    in_maps = []
    for core in range(N_CORES):
        h0 = H_PER_CORE * core
        blocks = []
        pe_rows = []
        for h in (h0, h0 + 1):
            blk = wq_b[192 * h : 192 * (h + 1)]
            blocks.append(blk[:NOPE])
            pe_rows.append(blk[NOPE:][perm])
        wqb_re = np.concatenate(blocks + pe_rows, 0)       # [384, QLR]
        # kv_a_ln folded into the up-projection weights (latent-dim scale)
        wukt = np.stack(
            [np.ascontiguousarray((kvb[h, :NOPE, :] * kvln[None, :]).T)
             for h in (h0, h0 + 1)]
        )                                                  # [2, 512, 128]
        wuv2 = np.concatenate(
            [(kvb[h, NOPE:, :] * kvln[None, :]).T for h in (h0, h0 + 1)],
            axis=1,
        )                                                  # [512, 256]
        wo_c = np.ascontiguousarray(wo[:, VD * h0 : VD * (h0 + 2)].T)   # [256, HID]
        sl = slice(SHW * core, SHW * (core + 1))
        in_maps.append({
            **shared,
            "hid_own": _bf16(hid_T[:, sl]),
            "wqb_t": _bf16(wqb_re.T),
            "wukt": _bf16(wukt),
            "wuv2": _bf16(np.ascontiguousarray(wuv2)),
            "wo_t": _bf16(wo_c),
            "cos2o": _bf16(cos2[:, sl]),
            "sin2o": _bf16(sin2n[:, sl]),
        })
    return in_maps


def kernel(**inputs):
    from concourse.bass_utils import run_bass_kernel_spmd

    nc = _get_program()
    in_maps = _host_inputs(**inputs)
    res = run_bass_kernel_spmd(nc, in_maps, core_ids=list(range(N_CORES)))
    acc = None
    for r in res.results:
        o = np.asarray(r["out_t"], dtype=np.float32)
        acc = o if acc is None else acc + o
    out = np.ascontiguousarray(acc.T)[None]                # [1, S, HID]
    return out.astype(np.float32)


# revision 39
# speedup vs baseline: 1.1143x; 1.1143x over previous
"""MLA (DeepSeek-style multi-head latent attention) forward on 8 TRN2 NeuronCores.

Sharding: BOTH big replicated projections are sharded over sequence and
AllGathered: the shared latent ckv projection (each core computes its own
256-seq slice of [640, S] and the raw bf16 latent + locally-roped k_pe +
sum-of-squares rows are gathered first on the CC queue), and the q_down
projection (as before, split into two gathers riding mid/late phase L).
Attention and the output projection are tensor-parallel over heads (2/core);
partial wo outputs are summed on host.

RMS folding: kv_a_ln is folded into the k/v up-projection weights on host.
The latent rms scale rk (per k position) is folded into the EXP activation's
per-partition scale (scoresT has k on partitions) and into the V PSUM-drain
copy (V has k on partitions); roped k_pe is pre-multiplied by the rms on the
shard side so the exp-scale fold leaves it unscaled. No ksn materialization,
no rms broadcast matmuls on the k side.

All matmul operands are bf16 (FWL weight loads, half DMA); accumulation fp32
in PSUM. Softmax denominators accumulate on DVE and reduce across partitions
via a ones-matmul + broadcast-matmul (replacing gpsimd partition_all_reduce,
which blocked the in-order PE queue ~7us per chunk boundary). Score rope MMs
(K=64) for the two heads are emitted back-to-back so they run concurrently
in different PE row strips.

Pipeline per core (S=2048; 4 seq-chunks of 512 for full-S phases):
  C: local ckv shard [640, 256] + ssq + local k_pe rope/rms -> AG2 (first on
     the CC queue; mat depends on it).
  L: local q_down shard (as before) -> AG1a (after 6 l-tiles), AG1b (+ssq).
  M: post-AG2: DMA gathered latent/kpe/ssq into SBUF, rk prep [128,16];
     per-head k_nope (raw) and V (rk-scaled in the ACT drain).
  B: post-AG1 wq_b (12 l-tile PSUM accumulation), r_q fold, q rope.
  A: per k-block: nope MMs both heads then concurrent rope MMs; exp with
     per-partition scale SCALE*rk; causal mask on diagonal; ctx accumulation;
     denominators via ones-MM reduce + bc-MM at chunk ends; wo groups of the
     previous chunk interleaved into the score stream; bf16 outT.
Host: sum the 8 partial outT in fp32, transpose -> [1, S, HID].
"""

import numpy as np

S = 2048
HID = 2048
QLR = 1536
H_PER_CORE = 2
N_CORES = 8
NOPE = 128
ROPE = 64
VD = 128
KVL = 512
EPS = 1e-6
THETA = 10000.0
SCALE = float((NOPE + ROPE) ** -0.5)
NC_ = 4            # seq chunks
CW = 512           # chunk width
SHW = S // N_CORES  # 256-wide local shard
KB = S // 128      # 16 k-blocks
NLT = QLR // 128   # 12 l-tiles
CKR = 5 * 128 + 2  # ckv shard rows: 512 latent + 128 roped kpe + 2 ssq(f32)


def _bf16(a):
    import ml_dtypes
    return np.ascontiguousarray(np.asarray(a, np.float32)).astype(ml_dtypes.bfloat16)


def _build_program():
    import concourse.mybir as mybir
    import concourse.tile as tile
    from concourse import bacc

    f32 = mybir.dt.float32
    f32r = mybir.dt.float32r
    bf16 = mybir.dt.bfloat16
    AF = mybir.ActivationFunctionType
    OP = mybir.AluOpType

    nc = bacc.Bacc("TRN2", target_bir_lowering=False, num_devices=N_CORES)

    hid_own = nc.dram_tensor("hid_own", [HID, SHW], bf16, kind="ExternalInput")
    wqa_t = nc.dram_tensor("wqa_t", [HID, QLR], bf16, kind="ExternalInput")
    wqb_t = nc.dram_tensor("wqb_t", [QLR, 384], bf16, kind="ExternalInput")
    wkv_t = nc.dram_tensor("wkv_t", [HID, 640], bf16, kind="ExternalInput")
    wukt_d = nc.dram_tensor("wukt", [H_PER_CORE, KVL, NOPE], bf16,
                            kind="ExternalInput")
    wuv2_d = nc.dram_tensor("wuv2", [KVL, H_PER_CORE * VD], bf16,
                            kind="ExternalInput")
    wo_t = nc.dram_tensor("wo_t", [H_PER_CORE * VD, HID], bf16, kind="ExternalInput")
    cos2_d = nc.dram_tensor("cos2", [128, S], bf16, kind="ExternalInput")
    sin2n_d = nc.dram_tensor("sin2n", [128, S], bf16, kind="ExternalInput")
    cos2o_d = nc.dram_tensor("cos2o", [128, SHW], bf16, kind="ExternalInput")
    sin2o_d = nc.dram_tensor("sin2o", [128, SHW], bf16, kind="ExternalInput")
    swappb_d = nc.dram_tensor("swappb", [128, 128], bf16, kind="ExternalInput")
    maskt_d = nc.dram_tensor("maskt", [128, 128], bf16, kind="ExternalInput")
    out_t = nc.dram_tensor("out_t", [HID, S], bf16, kind="ExternalOutput")

    with tile.TileContext(nc) as tc:
        with (
            tc.tile_pool(name="stats", bufs=1) as stats,
            tc.tile_pool(name="dram", bufs=1, space="DRAM") as dram,
            tc.tile_pool(name="consts", bufs=1) as consts,
            tc.tile_pool(name="resid", bufs=1) as resid,
            tc.tile_pool(name="bqp", bufs=1) as bqp,
        ):
            ones_p = stats.tile([128, 1], f32r)
            nc.vector.memset(ones_p.bitcast(f32), 1.0)
            ones_row = stats.tile([1, 128], f32r)
            nc.vector.memset(ones_row.bitcast(f32), 1.0)
            eps_sb = stats.tile([1, 1], f32)
            nc.vector.memset(eps_sb, EPS)
            eps_p = stats.tile([128, 1], f32)
            nc.vector.memset(eps_p, EPS)

            ckb_shard = dram.tile([CKR, SHW], bf16)
            ckb_gath = dram.tile([N_CORES * CKR, SHW], bf16, addr_space="Shared")
            qdn_shard1 = dram.tile([6 * 128, SHW], bf16)
            qdn_shard2 = dram.tile([6 * 128 + 1, SHW], bf16)
            qdn_full1 = dram.tile([N_CORES * 6 * 128, SHW], bf16,
                                  addr_space="Shared")
            qdn_full2 = dram.tile([N_CORES * (6 * 128 + 1), SHW], bf16,
                                  addr_space="Shared")

            wukt_sb = consts.tile([128, H_PER_CORE, 4, NOPE], bf16)
            wuv2_sb = consts.tile([128, 4, H_PER_CORE * VD], bf16)
            cos2_sb = consts.tile([128, S], bf16)
            sin2n_sb = consts.tile([128, S], bf16)
            cos2o_sb = consts.tile([128, SHW], bf16)
            sin2o_sb = consts.tile([128, SHW], bf16)
            swappb_sb = consts.tile([128, 128], bf16)
            maskt_sb = consts.tile([128, 128], bf16)
            wqb_sb = consts.tile([128, NLT, 384], bf16)
            wo_sb = consts.tile([128, H_PER_CORE, HID], bf16)

            kpe = resid.tile([128, S], bf16)          # roped+rms k_peT (2 copies)
            kn_sb = resid.tile([128, H_PER_CORE, S], bf16)  # per-head raw k_nopeT
            v_sb = resid.tile([128, KB, H_PER_CORE * VD], bf16)  # V, rk-scaled
            ctxa = resid.tile([128, H_PER_CORE, S], bf16)
            ckb_sb = resid.tile([128, 4, S], bf16)    # gathered raw latent
            rkp = resid.tile([128, KB], f32)          # 1/rms per k position
            rkp_e = resid.tile([128, KB], f32)        # SCALE/rms (exp scale)
            qdn_all = bqp.tile([128, NLT, N_CORES, SHW], bf16)
            ssq_all = bqp.tile([1, N_CORES, SHW], bf16)

            # ========== Phase C: local ckv shard -> AG2 ==========
            hidp_ctx = tc.tile_pool(name="hidp", bufs=1)
            hidp = hidp_ctx.__enter__()
            wkvp_ctx = tc.tile_pool(name="wkvp", bufs=1)
            wkvp = wkvp_ctx.__enter__()
            cl_ctx = tc.tile_pool(name="clwork", bufs=1)
            clw = cl_ctx.__enter__()
            with (
                tc.tile_pool(name="cwork", bufs=2) as cwork,
                tc.tile_pool(name="cpsum", bufs=2, space="PSUM") as cpsum,
                tc.tile_pool(name="cp1", bufs=1, space="PSUM") as cp1,
                tc.tile_pool(name="cmisc", bufs=1, space="PSUM") as cmisc,
            ):
                hid_own_sb = hidp.tile([128, KB, SHW], bf16)
                hid_src = hid_own.ap().rearrange("(kt p) s -> p kt s", p=128)
                for q in range(4):
                    ks = slice(4 * q, 4 * q + 4)
                    nc.sync.dma_start(hid_own_sb[:, ks, :], hid_src[:, ks, :])
                wkv_sb = wkvp.tile([128, KB, 640], bf16, tag="wkv")
                for dt in range(5):
                    nc.sync.dma_start(
                        wkv_sb[:, :, 128 * dt : 128 * (dt + 1)],
                        wkv_t.ap()[:, 128 * dt : 128 * (dt + 1)].rearrange(
                            "(kt p) m -> p kt m", p=128
                        ),
                    )
                nc.sync.dma_start(cos2o_sb, cos2o_d.ap())
                nc.sync.dma_start(sin2o_sb, sin2o_d.ap())
                nc.sync.dma_start(swappb_sb, swappb_d.ap())

                ps_cssq = cp1.tile([1, SHW], f32, tag="cssq")
                ckl = [None] * 5
                ck_ps = [None] * 5

                def emit_csq(dt):
                    # square the bf16 copy on DVE (ACT's activation-table
                    # warmup stalls the chain early in the kernel)
                    sq = cwork.tile([128, SHW], f32r, tag="csq")
                    nc.vector.tensor_tensor(sq, ckl[dt], ckl[dt], OP.mult)
                    nc.tensor.matmul(
                        ps_cssq, ones_p, sq, start=(dt == 0), stop=(dt == 3)
                    )

                for dt in range(5):
                    ps_ck = cpsum.tile([128, SHW], f32, tag="ck")
                    for kt in range(KB):
                        nc.tensor.matmul(
                            ps_ck,
                            wkv_sb[:, kt, 128 * dt : 128 * (dt + 1)],
                            hid_own_sb[:, kt, :],
                            start=(kt == 0),
                            stop=(kt == KB - 1),
                        )
                    ck_ps[dt] = ps_ck
                    if dt >= 2:
                        emit_csq(dt - 2)
                    ckl[dt] = clw.tile([128, SHW], bf16, tag=f"ckl{dt}",
                                       name=f"ckl{dt}")
                    nc.vector.tensor_copy(ckl[dt], ps_ck)
                    if dt < 4:
                        nc.gpsimd.dma_start(
                            ckb_shard[128 * dt : 128 * (dt + 1), :], ckl[dt]
                        )
                emit_csq(3)

                # local rms row + roped, rms-scaled k_pe
                msk = clw.tile([1, SHW], f32, tag="msk")
                nc.scalar.activation(
                    msk, ps_cssq, AF.Sqrt, scale=1.0 / KVL, bias=eps_sb
                )
                msk_r = clw.tile([1, SHW], f32r, tag="mskr")
                nc.vector.tensor_copy(msk_r, msk)
                ssql = clw.tile([1, SHW], f32, tag="ssql")
                nc.vector.tensor_copy(ssql, ps_cssq)

                ps_sw = cmisc.tile([128, SHW], f32, tag="cm", name="ps_csw")
                nc.tensor.matmul(ps_sw, swappb_sb, ckl[4], start=True, stop=True)
                t1 = cwork.tile([128, SHW], f32, tag="ct1")
                nc.vector.tensor_tensor(t1, ckl[4], cos2o_sb, OP.mult)
                t2 = cwork.tile([128, SHW], f32, tag="ct2")
                nc.vector.tensor_tensor(t2, ps_sw, sin2o_sb, OP.mult)
                kroped = cwork.tile([128, SHW], f32, tag="ckro")
                nc.vector.tensor_tensor(kroped, t1, t2, OP.add)
                ps_bc = cmisc.tile([128, SHW], f32, tag="cm", name="ps_cbc")
                nc.tensor.matmul(ps_bc, ones_row, msk_r, start=True, stop=True)
                kpe_l = clw.tile([128, SHW], bf16, tag="kpel")
                nc.vector.tensor_tensor(kpe_l, kroped, ps_bc, OP.mult)

                nc.gpsimd.dma_start(ckb_shard[512:640, :], kpe_l)
                # ssq shipped as a hi/lo bf16 pair (~f32 precision, and all
                # valid bf16 values -- a raw f32 bitcast trips NaN checks)
                ssq_hi = clw.tile([1, SHW], bf16, tag="ssqhi")
                nc.vector.tensor_copy(ssq_hi, ssql)
                ssq_lo = clw.tile([1, SHW], bf16, tag="ssqlo")
                nc.vector.tensor_tensor(ssq_lo, ssql, ssq_hi, OP.subtract)
                nc.gpsimd.dma_start(ckb_shard[640:641, :], ssq_hi)
                nc.gpsimd.dma_start(ckb_shard[641:642, :], ssq_lo)
                nc.gpsimd.collective_compute(
                    "AllGather",
                    mybir.AluOpType.bypass,
                    replica_groups=[list(range(N_CORES))],
                    ins=[ckb_shard.opt()],
                    outs=[ckb_gath.opt()],
                )

            # ========== Phase L: local q_down shard -> split AllGather =====
            with (
                tc.tile_pool(name="wqap", bufs=2) as wqap,
                tc.tile_pool(name="lwork", bufs=2) as lwork,
                tc.tile_pool(name="lbig", bufs=1) as lbig,
                tc.tile_pool(name="lpsum", bufs=2, space="PSUM") as lpsum,
                tc.tile_pool(name="lpsum1", bufs=1, space="PSUM") as lpsum1,
            ):
                wqa_g = [None] * 3
                for lg in range(3):
                    wqa_g[lg] = wqap.tile([128, KB, 512], bf16, tag="wqa",
                                          name=f"wqa{lg}")
                    wqa_src = wqa_t.ap()[:, 512 * lg : 512 * (lg + 1)].rearrange(
                        "(kt p) m -> p kt m", p=128
                    )
                    for q in range(4):
                        ks = slice(4 * q, 4 * q + 4)
                        nc.sync.dma_start(
                            wqa_g[lg][:, ks, :], wqa_src[:, ks, :]
                        )
                # late constants, behind the phase-C/L critical DMAs
                nc.sync.dma_start(
                    wukt_sb, wukt_d.ap().rearrange("h (lt p) n -> p h lt n", p=128)
                )
                nc.sync.dma_start(
                    wuv2_sb, wuv2_d.ap().rearrange("(lt p) v -> p lt v", p=128)
                )
                for q in range(2):
                    hs_ = slice(1024 * q, 1024 * (q + 1))
                    nc.sync.dma_start(cos2_sb[:, hs_], cos2_d.ap()[:, hs_])
                    nc.sync.dma_start(sin2n_sb[:, hs_], sin2n_d.ap()[:, hs_])
                nc.sync.dma_start(maskt_sb, maskt_d.ap())
                wqb_src = wqb_t.ap().rearrange("(li p) m -> p li m", p=128)
                for q in range(3):
                    ls = slice(4 * q, 4 * q + 4)
                    nc.sync.dma_start(wqb_sb[:, ls, :], wqb_src[:, ls, :])
                wo_src = wo_t.ap().rearrange("(h p) m -> p h m", p=128)
                for q in range(4):
                    hs_ = slice(512 * q, 512 * (q + 1))
                    nc.sync.dma_start(wo_sb[:, :, hs_], wo_src[:, :, hs_])

                qdn_sb = lbig.tile([128, NLT, SHW], bf16)
                ssqb = stats.tile([1, SHW], bf16)
                ps_ssq = lpsum1.tile([1, SHW], f32, tag="ssq")

                def emit_ssq(li):
                    sq = lwork.tile([128, SHW], f32r, tag="sq")
                    nc.vector.tensor_tensor(
                        sq, qdn_sb[:, li, :], qdn_sb[:, li, :], OP.mult
                    )
                    nc.tensor.matmul(
                        ps_ssq, ones_p, sq, start=(li == 0), stop=(li == NLT - 1)
                    )

                for li in range(NLT):
                    ps_qd = lpsum.tile([128, SHW], f32, tag="qd",
                                       name=f"ps_qd{li % 2}")
                    for kt in range(KB):
                        nc.tensor.matmul(
                            ps_qd,
                            wqa_g[li // 4][:, kt, 128 * (li % 4) : 128 * (li % 4 + 1)],
                            hid_own_sb[:, kt, :],
                            start=(kt == 0),
                            stop=(kt == KB - 1),
                        )
                    nc.vector.tensor_copy(qdn_sb[:, li, :], ps_qd)
                    if li > 1:
                        emit_ssq(li - 2)
                    if li == 5:
                        nc.gpsimd.dma_start(
                            qdn_shard1.rearrange("(li p) s -> p li s", p=128),
                            qdn_sb[:, 0:6, :],
                        )
                        nc.gpsimd.collective_compute(
                            "AllGather",
                            mybir.AluOpType.bypass,
                            replica_groups=[list(range(N_CORES))],
                            ins=[qdn_shard1.opt()],
                            outs=[qdn_full1.opt()],
                        )
                emit_ssq(NLT - 2)
                emit_ssq(NLT - 1)
                nc.vector.tensor_copy(ssqb, ps_ssq)
                nc.gpsimd.dma_start(
                    qdn_shard2[0 : 6 * 128, :].rearrange(
                        "(li p) s -> p li s", p=128
                    ),
                    qdn_sb[:, 6:12, :],
                )
                nc.gpsimd.dma_start(qdn_shard2[6 * 128 : 6 * 128 + 1, :], ssqb)
                nc.gpsimd.collective_compute(
                    "AllGather",
                    mybir.AluOpType.bypass,
                    replica_groups=[list(range(N_CORES))],
                    ins=[qdn_shard2.opt()],
                    outs=[qdn_full2.opt()],
                )
                src1 = qdn_full1.rearrange(
                    "(r li p) s -> p li r s", p=128, li=6
                )
                for li2 in range(6):
                    nc.gpsimd.dma_start(
                        qdn_all[:, li2, :, :], src1[:, li2, :, :]
                    )
                src2 = qdn_full2.rearrange("(r x) s -> x r s", x=6 * 128 + 1)
                nc.gpsimd.dma_start(ssq_all, src2[6 * 128 : 6 * 128 + 1, :, :])
                for li in range(6, NLT):
                    nc.gpsimd.dma_start(
                        qdn_all[:, li, :, :],
                        src2[128 * (li - 6) : 128 * (li - 5), :, :],
                    )
            wqa_g = None
            cl_ctx.__exit__(None, None, None)
            wkvp_ctx.__exit__(None, None, None)

            # ====== Phase M: unpack AG2, rk prep, k_nope + V materialize ====
            with (
                tc.tile_pool(name="mwork", bufs=2) as mwork,
                tc.tile_pool(name="kpsum", bufs=2, space="PSUM") as kpsum,
                tc.tile_pool(name="vpsum", bufs=2, space="PSUM") as vpsum,
            ):
                gview = ckb_gath.rearrange("(r x) c -> x r c", x=CKR)
                for lt in range(4):
                    nc.scalar.dma_start(
                        ckb_sb[:, lt, :].rearrange("p (r c) -> p r c", r=N_CORES),
                        gview[128 * lt : 128 * (lt + 1), :, :],
                    )
                nc.scalar.dma_start(
                    kpe.rearrange("p (r c) -> p r c", r=N_CORES),
                    gview[512:640, :, :],
                )
                # gathered ssq rows land straight in [128, KB] layout
                # (k-position on partitions): kb = 2*rank + x, p = c mod 128
                rk_hi = mwork.tile([128, KB], bf16, tag="rkhi")
                rk_lo = mwork.tile([128, KB], bf16, tag="rklo")
                for dst, srow in ((rk_hi, 640), (rk_lo, 641)):
                    dv = dst.rearrange("p (r x) -> p r x", x=2)
                    sv = gview[srow : srow + 1, :, :].rearrange(
                        "a r (x p) -> p r x", p=128
                    )
                    for x in range(2):
                        nc.scalar.dma_start(dv[:, :, x], sv[:, :, x])
                rkp_raw = mwork.tile([128, KB], f32, tag="rkpr")
                nc.vector.tensor_tensor(rkp_raw, rk_hi, rk_lo, OP.add)
                rms_p = mwork.tile([128, KB], f32, tag="rmsp")
                nc.scalar.activation(
                    rms_p, rkp_raw, AF.Sqrt, scale=1.0 / KVL, bias=eps_p
                )
                nc.vector.reciprocal_approx_fast(out=rkp, in_=rms_p)
                nc.vector.tensor_scalar_mul(rkp_e, rkp, SCALE)

                for h in range(H_PER_CORE):
                    for c in range(NC_):
                        cs = slice(CW * c, CW * (c + 1))
                        ps_k = kpsum.tile([128, CW], f32, tag="kn")
                        for lt in range(4):
                            nc.tensor.matmul(
                                ps_k,
                                wukt_sb[:, h, lt, :],
                                ckb_sb[:, lt, cs],
                                start=(lt == 0),
                                stop=(lt == 3),
                            )
                        nc.vector.tensor_copy(kn_sb[:, h, cs], ps_k)
                for b in range(KB):
                    ps_v = vpsum.tile([128, H_PER_CORE * VD], f32, tag="v")
                    for lt in range(4):
                        nc.tensor.matmul(
                            ps_v,
                            ckb_sb[:, lt, 128 * b : 128 * (b + 1)],
                            wuv2_sb[:, lt, :],
                            start=(lt == 0),
                            stop=(lt == 3),
                        )
                    nc.scalar.activation(
                        v_sb[:, b, :], ps_v, AF.Copy, scale=rkp[:, b : b + 1]
                    )

            # ===== Phase B: post-gather wq_b + q rope =====
            bres_ctx = tc.tile_pool(name="bres", bufs=1)
            bres = bres_ctx.__enter__()
            qtr = bres.tile([128, 3, S], bf16)      # post-gather q (r_q folded)
            qspe = bres.tile([128, S], bf16)        # roped q_peT
            with (
                tc.tile_pool(name="bwork", bufs=2) as bwork,
                tc.tile_pool(name="bpsum", bufs=3, space="PSUM") as bpsum,
                tc.tile_pool(name="bmisc", bufs=1, space="PSUM") as bmisc,
            ):
                rqf = bwork.tile([1, S], f32, tag="rqf")
                nc.scalar.activation(
                    rqf, ssq_all, AF.Sqrt, scale=1.0 / QLR, bias=eps_sb
                )
                nc.vector.reciprocal_approx_fast(out=rqf, in_=rqf)
                rqf_r = bwork.tile([1, S], f32r, tag="rqfr")
                nc.vector.tensor_copy(rqf_r, rqf)
                for c in range(NC_):
                    cs = slice(CW * c, CW * (c + 1))
                    ps_rq = bmisc.tile([128, CW], f32, tag="misc",
                                       name="ps_rq")
                    nc.tensor.matmul(
                        ps_rq, ones_row, rqf_r[:, cs], start=True, stop=True
                    )
                    rqsb = bwork.tile([128, CW], f32, tag="rqsb")
                    nc.vector.tensor_copy(rqsb, ps_rq)
                    for dt in range(3):
                        ps_qt = bpsum.tile([128, CW], f32, tag="qt")
                        for li in range(NLT):
                            nc.tensor.matmul(
                                ps_qt,
                                wqb_sb[:, li, 128 * dt : 128 * (dt + 1)],
                                qdn_all[:, li, 2 * c : 2 * c + 2, :],
                                start=(li == 0),
                                stop=(li == NLT - 1),
                            )
                        nc.vector.tensor_tensor(
                            qtr[:, dt, cs], ps_qt, rqsb, OP.mult
                        )
                    # rope q_pe (both heads stacked)
                    ps_sw = bmisc.tile([128, CW], f32, tag="misc",
                                       name="ps_swq")
                    nc.tensor.matmul(
                        ps_sw, swappb_sb, qtr[:, 2, cs], start=True, stop=True
                    )
                    t1 = bwork.tile([128, CW], f32, tag="t1")
                    nc.vector.tensor_tensor(
                        t1, qtr[:, 2, cs], cos2_sb[:, cs], OP.mult
                    )
                    t2 = bwork.tile([128, CW], f32, tag="t2")
                    nc.vector.tensor_tensor(t2, ps_sw, sin2n_sb[:, cs], OP.mult)
                    nc.vector.tensor_tensor(qspe[:, cs], t1, t2, OP.add)

            # ================= Phase A: attention =================
            with (
                tc.tile_pool(name="accp", bufs=2) as accp,
                tc.tile_pool(name="attp", bufs=3) as attp,
                tc.tile_pool(name="obp", bufs=2) as obp,
                tc.tile_pool(name="arow", bufs=2) as arow,
                tc.tile_pool(name="abc", bufs=2) as abc,
                tc.tile_pool(name="aacc", bufs=1, space="PSUM") as aacc,
                tc.tile_pool(name="ascore", bufs=2, space="PSUM") as ascore,
                tc.tile_pool(name="ascr1", bufs=2, space="PSUM") as ascr1,
                tc.tile_pool(name="wpsum", bufs=2, space="PSUM") as wpsum,
            ):
                def emit_w_group(c, hg):
                    cs = slice(CW * c, CW * (c + 1))
                    ob4 = obp.tile([128, 4, CW], bf16, tag="ob4")
                    for hi in range(4):
                        ht = 4 * hg + hi
                        ps_o = wpsum.tile([128, CW], f32, tag="o",
                                          name=f"ps_o{hi % 2}")
                        for h in range(H_PER_CORE):
                            nc.tensor.matmul(
                                ps_o,
                                wo_sb[:, h, 128 * ht : 128 * (ht + 1)],
                                ctxa[:, h, cs],
                                start=(h == 0),
                                stop=(h == H_PER_CORE - 1),
                            )
                        # alternate PSUM drains between DVE and ACT (DVE is
                        # near-saturated in late chunks)
                        if hi % 2 == 0:
                            nc.vector.tensor_copy(ob4[:, hi, :], ps_o)
                        else:
                            nc.scalar.activation(ob4[:, hi, :], ps_o, AF.Copy)
                    out_dst = (
                        out_t.ap()[512 * hg : 512 * (hg + 1), cs]
                        .rearrange("(ht p) s -> p ht s", p=128)
                    )
                    nc.sync.dma_start(out_dst[:, 0:2, :], ob4[:, 0:2, :])
                    nc.sync.dma_start(out_dst[:, 2:4, :], ob4[:, 2:4, :])

                chunk_state = [None]  # (c, cs, ps_ctx, rec_r)

                def emit_den(c, cs, ps_ctx, dacc):
                    rec_r = []
                    for h in range(H_PER_CORE):
                        t_den = wpsum.tile([128, CW], f32, tag="o",
                                           name=f"t_den{h}")
                        nc.tensor.matmul(
                            t_den[0:1, :], ones_p, dacc[h], start=True, stop=True
                        )
                        rec = arow.tile([1, CW], f32, tag=f"rec{h}",
                                        name=f"rec{h}")
                        nc.vector.reciprocal_approx_fast(
                            out=rec, in_=t_den[0:1, :]
                        )
                        rec_fr = arow.tile([1, CW], f32r, tag=f"recr{h}",
                                           name=f"recr{h}")
                        nc.vector.tensor_copy(rec_fr, rec)
                        rec_r.append(rec_fr)
                    chunk_state[0] = (c, cs, ps_ctx, rec_r)

                def emit_epilogue():
                    c, cs, ps_ctx, rec_r = chunk_state[0]
                    for h in range(H_PER_CORE):
                        t_bc = wpsum.tile([128, CW], f32, tag="o",
                                          name=f"t_bc{h}")
                        nc.tensor.matmul(
                            t_bc, ones_row, rec_r[h], start=True, stop=True
                        )
                        bc_sb = abc.tile([128, CW], f32, tag=f"bc{h}",
                                         name=f"bc{h}")
                        nc.scalar.activation(bc_sb, t_bc, AF.Copy)
                        nc.vector.tensor_tensor(
                            ctxa[:, h, cs], ps_ctx[h], bc_sb, OP.mult
                        )
                    chunk_state[0] = None

                for c in range(NC_):
                    cs = slice(CW * c, CW * (c + 1))
                    nj = 4 * c + 4
                    ps_ctx = [
                        aacc.tile([128, CW], f32, tag=f"ctx{h}",
                                  name=f"ps_ctx{h}")
                        for h in range(H_PER_CORE)
                    ]
                    dacc = [
                        accp.tile([128, CW], f32r, tag=f"dacc{h}",
                                  name=f"dacc{h}")
                        for h in range(H_PER_CORE)
                    ]
                    atts = [[None, None] for _ in range(nj)]
                    offs = [0 if j < 4 * c else 128 * (j - 4 * c)
                            for j in range(nj)]

                    def emit_scores(j, c=c, nj=nj, offs=offs, atts=atts,
                                    dacc=dacc):
                        off = offs[j]
                        q0 = CW * c + off
                        qs = slice(q0, CW * (c + 1))
                        pools = [ascore, ascr1]
                        ps_s = [None, None]
                        for h in range(H_PER_CORE):
                            ps_s[h] = pools[h].tile([128, CW], f32,
                                                    tag=f"sc{h}",
                                                    name=f"ps_s{h}")
                            nc.tensor.matmul(
                                ps_s[h][:, off:],
                                kn_sb[:, h, 128 * j : 128 * (j + 1)],
                                qtr[:, h, qs],
                                start=True,
                                stop=False,
                            )
                        for h in range(H_PER_CORE):
                            # K=64 rope MMs: the two heads land in PE row
                            # strips 0-63 / 64-127 and run concurrently.
                            nc.tensor.matmul(
                                ps_s[h][:, off:],
                                kpe[64 * h : 64 * (h + 1),
                                    128 * j : 128 * (j + 1)],
                                qspe[64 * h : 64 * (h + 1), qs],
                                start=False,
                                stop=True,
                            )
                        for h in range(H_PER_CORE):
                            att = attp.tile([128, CW], bf16, tag=f"att{h}",
                                            name=f"att{h}")
                            nc.scalar.activation(
                                att[:, off:], ps_s[h][:, off:], AF.Exp,
                                scale=rkp_e[:, j : j + 1],
                            )
                            if j >= 4 * c:
                                nc.vector.tensor_tensor(
                                    att[:, off : off + 128],
                                    att[:, off : off + 128],
                                    maskt_sb,
                                    OP.mult,
                                )
                            if j == 0:
                                nc.vector.tensor_copy(dacc[h], att)
                            else:
                                nc.vector.tensor_tensor(
                                    dacc[h][:, off:],
                                    dacc[h][:, off:],
                                    att[:, off:],
                                    OP.add,
                                )
                            atts[j][h] = att

                    def emit_ctx(j, c=c, nj=nj, offs=offs, atts=atts,
                                 ps_ctx=ps_ctx):
                        off = offs[j]
                        for h in range(H_PER_CORE):
                            nc.tensor.matmul(
                                ps_ctx[h][:, off:],
                                v_sb[:, j, VD * h : VD * (h + 1)],
                                atts[j][h][:, off:],
                                start=(j == 0),
                                stop=(j == nj - 1),
                            )

                    emit_scores(0)
                    emit_scores(1)
                    if chunk_state[0] is not None:
                        emit_epilogue()
                    emit_ctx(0)
                    pending_w = (
                        [(c - 1, hg) for hg in range(4)] if c >= 1 else []
                    )
                    for j in range(2, nj):
                        emit_scores(j)
                        emit_ctx(j - 1)
                        if pending_w and j >= 3:
                            emit_w_group(*pending_w.pop(0))
                    emit_ctx(nj - 1)
                    while pending_w:
                        emit_w_group(*pending_w.pop(0))
                    emit_den(c, cs, ps_ctx, dacc)
                emit_epilogue()
                for hg in range(4):
                    emit_w_group(NC_ - 1, hg)
            bres_ctx.__exit__(None, None, None)
            hidp_ctx.__exit__(None, None, None)

    nc.finalize()
    return nc


_PROGRAM = None


def _get_program():
    global _PROGRAM
    if _PROGRAM is None:
        _PROGRAM = _build_program()
    return _PROGRAM


def _host_inputs(hidden_states, position_ids, wq_a, q_a_ln_w, wq_b, wkv_a,
                 kv_a_ln_w, wkv_b, wo):
    """Build the 8 per-core input maps."""
    hs = np.asarray(hidden_states, np.float32)[0]          # [S, HID]
    pos = np.asarray(position_ids)[0].astype(np.int64)     # [S]

    # rope tables (fp32, matching the reference)
    inv_freq = (1.0 / (THETA ** (np.arange(0, ROPE, 2, dtype=np.float32) / ROPE))).astype(np.float32)
    t = pos.astype(np.float32)
    freqs = np.outer(t, inv_freq).astype(np.float32)       # [S, 32]
    emb = np.concatenate([freqs, freqs], -1)               # [S, 64]
    cos = np.cos(emb).astype(np.float32)
    sin = np.sin(emb).astype(np.float32)
    cosT = np.ascontiguousarray(cos.T)                     # [64, S]
    sinT = np.ascontiguousarray(sin.T)
    sinTn = sinT.copy()
    sinTn[:32] = -sinTn[:32]                               # fold rotate_half sign
    cos2 = np.concatenate([cosT, cosT], 0)                 # [128, S]
    sin2n = np.concatenate([sinTn, sinTn], 0)

    perm = np.concatenate([np.arange(0, ROPE, 2), np.arange(1, ROPE, 2)])  # interleave

    # swap-halves permutation matrix (two independent 64 blocks)
    swapp = np.zeros((128, 128), np.float32)
    for m in range(128):
        base = (m // 64) * 64
        i = m % 64
        swapp[base + (i + 32) % 64, m] = 1.0

    maskt = np.triu(np.ones((128, 128), np.float32))

    wq_b = np.asarray(wq_b, np.float32) * np.asarray(q_a_ln_w, np.float32)[None, :]
    kvln = np.asarray(kv_a_ln_w, np.float32)
    kvb = np.asarray(wkv_b, np.float32).reshape(16, NOPE + VD, KVL)
    wkv_a = np.asarray(wkv_a, np.float32)
    wkv_rows = np.concatenate(
        [wkv_a[:KVL], wkv_a[KVL:][perm], wkv_a[KVL:][perm]], 0
    )                                                      # [640, HID]

    hid_T = np.ascontiguousarray(hs.T)                     # [HID, S]
    shared = {
        "wqa_t": _bf16(np.asarray(wq_a, np.float32).T),
        "wkv_t": _bf16(wkv_rows.T),
        "cos2": _bf16(cos2), "sin2n": _bf16(sin2n),
        "swappb": _bf16(swapp),
        "maskt": _bf16(maskt),
    }

    wo = np.asarray(wo, np.float32)
    in_maps = []
    for core in range(N_CORES):
        h0 = H_PER_CORE * core
        blocks = []
        pe_rows = []
        for h in (h0, h0 + 1):
            blk = wq_b[192 * h : 192 * (h + 1)]
            blocks.append(blk[:NOPE])
            pe_rows.append(blk[NOPE:][perm])
        wqb_re = np.concatenate(blocks + pe_rows, 0)       # [384, QLR]
        # kv_a_ln folded into the up-projection weights (latent-dim scale)
        wukt = np.stack(
            [np.ascontiguousarray((kvb[h, :NOPE, :] * kvln[None, :]).T)
             for h in (h0, h0 + 1)]
        )                                                  # [2, 512, 128]
        wuv2 = np.concatenate(
            [(kvb[h, NOPE:, :] * kvln[None, :]).T for h in (h0, h0 + 1)],
            axis=1,
        )                                                  # [512, 256]
        wo_c = np.ascontiguousarray(wo[:, VD * h0 : VD * (h0 + 2)].T)   # [256, HID]
        sl = slice(SHW * core, SHW * (core + 1))
        in_maps.append({
            **shared,
            "hid_own": _bf16(hid_T[:, sl]),
            "wqb_t": _bf16(wqb_re.T),
            "wukt": _bf16(wukt),
            "wuv2": _bf16(np.ascontiguousarray(wuv2)),
            "wo_t": _bf16(wo_c),
            "cos2o": _bf16(cos2[:, sl]),
            "sin2o": _bf16(sin2n[:, sl]),
        })
    return in_maps


def kernel(**inputs):
    from concourse.bass_utils import run_bass_kernel_spmd

    nc = _get_program()
    in_maps = _host_inputs(**inputs)
    res = run_bass_kernel_spmd(nc, in_maps, core_ids=list(range(N_CORES)))
    acc = None
    for r in res.results:
        o = np.asarray(r["out_t"], dtype=np.float32)
        acc = o if acc is None else acc + o
    out = np.ascontiguousarray(acc.T)[None]                # [1, S, HID]
    return out.astype(np.float32)
